# revision 10
# baseline (speedup 1.0000x reference)
"""4-bit grouped-quant linear (BitBLAS-style) on 8 TRN2 NeuronCores.

y[m,n] = sum_k x[m,k] * (q[n,k] - zeros[n,g(k)]) * scales[n,g(k)] + bias[n]

Sharding: column-parallel (shard out_features N across 8 cores, replicate x).

Per core (N_shard = 1376), everything in [k, n] layout (host pre-transposes and
bit-packs the quantized weights — pure relayout; all math is on-device):
  - qsr[:, t, 0, :] holds packed nibble words (two packed bytes
    row-interleaved by the host), qsr[:, t, 1, :] the fp16 scales table
    replicated across the four 32-partition group bands (bit-viewed as u16 so
    both ride ONE DMA per k-tile; completion receipts are ~2us each, so fewer
    bigger DMAs win).  One [128, NS] word tile yields 4 q-planes
    (k = 4i+r) via shift/and ops; W'_r = q_r * s in fp16.
  - Dep-less warm-up matmuls run during the DMA lead-in to open the PE HAM
    clock gate before the real matmul stream arrives (any PE-idle gap >3.4us
    re-throttles the clock to half rate).
  - zero-points and bias fold into a rank-33 correction matmul:
        y = x @ (q*s)^T - sum_g zs[n,g] * t_g[m] + bias[n]
    with t_g[m] = sum_{k in g} x[m,k] via one indicator matmul per k-tile on
    xsum = sum_r x_plane_r (the adds run on the otherwise-idle GpSimd).
  - Main matmuls: lhsT = x^T plane tiles (stationary), rhs = W' tiles,
    PSUM-accumulated over 32 (t, r) k-tiles + the rank-33 correction.
  - Pipeline: the m-tiles 0+1 main matmuls are interleaved into the per-t
    dequant loop (6 PSUM banks + tps + warmup = 8), so the PE does real work
    while the k-tiles stream in from HBM; m-tiles 2+3 run after from SBUF.
  - Input DMA alternates between the two HWDGE queues (sync/scalar) per
    k-tile; the last m-tile's output store is split across both queues with
    the small chunk corrected/copied first to shorten the tail.
"""

import numpy as np
from contextlib import ExitStack

M, K, N, G = 512, 4096, 11008, 128
NCORES = 8
NS = N // NCORES          # 1376 out-features per core
NT = 8                    # uint16 word tiles (each: 128 partitions x 4 planes)
R = 4                     # nibble planes per word
MT = M // 128             # 4 m-tiles
NCHUNKS = [(0, 512), (512, 512), (1024, 352)]
NWARM = 13


def x_plane(xe_sb, t, r):
    return xe_sb[:, t, r * M:(r + 1) * M]


def build_bass():
    import concourse.mybir as mybir
    import concourse.tile as tile
    from concourse import bacc

    f16 = mybir.dt.float16
    f32 = mybir.dt.float32
    u16 = mybir.dt.uint16
    Alu = mybir.AluOpType

    nc = bacc.Bacc(None, target_bir_lowering=False)

    qsr = nc.declare_dram_parameter("qsr", [128, NT, 2, NS], u16, isOutput=False)
    xe = nc.declare_dram_parameter("xe", [128, NT, R * M + 32], f16, isOutput=False)
    sT32 = nc.declare_dram_parameter("sT32", [32, NS], f32, isOutput=False)
    zT32 = nc.declare_dram_parameter("zT32", [32, NS], f32, isOutput=False)
    biasr = nc.declare_dram_parameter("biasr", [1, NS], f32, isOutput=False)
    y = nc.declare_dram_parameter("y", [M, NS], f32, isOutput=True)

    with tile.TileContext(nc) as tc, ExitStack() as ctx:
        persist = ctx.enter_context(tc.tile_pool(name="persist", bufs=1))
        qpool = ctx.enter_context(tc.tile_pool(name="qpool", bufs=2))
        ypool = ctx.enter_context(tc.tile_pool(name="ypool", bufs=2))
        pspool = ctx.enter_context(tc.tile_pool(name="pspool", bufs=6, space="PSUM"))
        tpspool = ctx.enter_context(tc.tile_pool(name="tpspool", bufs=1, space="PSUM"))
        wupool = ctx.enter_context(tc.tile_pool(name="wupool", bufs=1, space="PSUM"))

        # ---- PE warm-up: dep-less matmuls keep the HAM clock gate open
        # while input DMAs land (PE is idle here anyway) --------------------
        wu_sb = persist.tile([128, 512], f16)
        nc.vector.memset(wu_sb, 0.0)
        wu_ps = wupool.tile([128, 512], f32)
        for i in range(NWARM):
            nc.tensor.matmul(wu_ps, wu_sb[:, :128], wu_sb,
                             start=True, stop=True, skip_group_check=True)

        # ---- per-tile: DMA, dequant, indicator + m-tile 0/1 main matmuls --
        st32_sb = persist.tile([32, NS], f32)
        zt32_sb = persist.tile([32, NS], f32)
        bias_sb = persist.tile([1, NS], f32)
        w4 = persist.tile([128, NT, R, NS], f16)
        xe_sb = persist.tile([128, NT, R * M + 32], f16)
        tps = tpspool.tile([32, M], f32)

        ps01 = [pspool.tile([128, 512], f32, tag="ps", name=f"ps_{mi}_{i}")
                for mi in range(2) for i in range(len(NCHUNKS))]
        HALF = 688
        for t in range(NT):
            qs = qpool.tile([128, 2, NS], u16, tag="qs", name=f"qs{t}")
            qt = qs[:, 0, :]
            sr = qs[:, 1, :].bitcast(f16)
            eng_q = nc.sync if t % 2 == 0 else nc.scalar
            eng_x = nc.scalar if t % 2 == 0 else nc.sync
            if t == 0:
                # halves ride both queues so dequant starts ~2us earlier
                nc.sync.dma_start(out=qs[:, :, 0:HALF], in_=qsr[:, 0, :, 0:HALF])
                nc.scalar.dma_start(out=qs[:, :, HALF:NS],
                                    in_=qsr[:, 0, :, HALF:NS])
            else:
                eng_q.dma_start(out=qs, in_=qsr[:, t, :, :])
            eng_x.dma_start(out=xe_sb[:, t, :], in_=xe[:, t, :])

            # dequant: w_r = ((qt >> 4r) & 15) * s
            pl = [qpool.tile([128, NS], u16, tag=f"pl{r}", name=f"pl{r}_{t}")
                  for r in range(R)]
            shifts = [(15, None, Alu.bitwise_and, None),
                      (4, 15, Alu.logical_shift_right, Alu.bitwise_and),
                      (8, 15, Alu.logical_shift_right, Alu.bitwise_and),
                      (12, None, Alu.logical_shift_right, None)]
            halves = [(0, HALF), (HALF, NS - HALF)] if t == 0 else [(0, NS)]
            for h0, hsz in halves:
                for r in range(R):
                    s1, s2, o1, o2 = shifts[r]
                    if o2 is None:
                        nc.vector.tensor_scalar(pl[r][:, h0:h0 + hsz],
                                                qt[:, h0:h0 + hsz], s1, None, o1)
                    else:
                        nc.vector.tensor_scalar(pl[r][:, h0:h0 + hsz],
                                                qt[:, h0:h0 + hsz], s1, s2, o1, o2)
                    nc.vector.tensor_tensor(out=w4[:, t, r, h0:h0 + hsz],
                                            in0=pl[r][:, h0:h0 + hsz],
                                            in1=sr[:, h0:h0 + hsz], op=Alu.mult)

            for r in range(R):
                nc.tensor.matmul(tps, xe_sb[:, t, R * M:R * M + 32],
                                 x_plane(xe_sb, t, r),
                                 start=(t == 0 and r == 0),
                                 stop=(t == NT - 1 and r == R - 1),
                                 skip_group_check=True)

            if t == 0:
                # chunk 0 lives in the first half; run all its mains first
                order = [(r, mi, 0) for r in range(R) for mi in range(2)] +                         [(r, mi, nci) for r in range(R) for mi in range(2)
                         for nci in (1, 2)]
            else:
                order = [(r, mi, nci) for r in range(R) for mi in range(2)
                         for nci in range(len(NCHUNKS))]
            for r, mi, nci in order:
                n0, nsz = NCHUNKS[nci]
                ms = slice(mi * 128, (mi + 1) * 128)
                nc.tensor.matmul(ps01[3 * mi + nci][:, :nsz],
                                 x_plane(xe_sb, t, r)[:, ms],
                                 w4[:, t, r, n0:n0 + nsz],
                                 start=(t == 0 and r == 0), stop=False,
                                 skip_group_check=True)

        # small tables ride the idle SWDGE queue (keeps receipts off the
        # sync/scalar chains); only needed for the correction matmuls
        nc.gpsimd.dma_start(out=st32_sb, in_=sT32[:, :])
        nc.gpsimd.dma_start(out=zt32_sb, in_=zT32[:, :])
        nc.gpsimd.dma_start(out=bias_sb, in_=biasr[:, :])

        # zs33: rows 0..31 = zeros*scales (fp32 mult -> fp16), row 32 = -bias
        zs33 = persist.tile([33, NS], f16)
        nc.vector.tensor_tensor(out=zs33[0:32, :], in0=zt32_sb, in1=st32_sb,
                                op=Alu.mult)
        nc.vector.tensor_scalar(zs33[32:33, :], bias_sb, -1.0, None, Alu.mult)

        # tT33: rows 0..31 = -t_g[m], row 32 = -1
        tT33 = persist.tile([33, M], f16)
        nc.scalar.copy(tT33[0:32, :], tps)
        nc.vector.memset(tT33[32:33, :], -1.0)

        # ---- finish m-tiles 0/1: correction + copy-out + store ------------
        for mi in range(2):
            ms = slice(mi * 128, (mi + 1) * 128)
            y_sb = ypool.tile([128, NS], f32, tag="ysb", name=f"ysb{mi}")
            for nci, (n0, nsz) in enumerate(NCHUNKS):
                nc.tensor.matmul(ps01[3 * mi + nci][:, :nsz], tT33[:, ms],
                                 zs33[:, n0:n0 + nsz], start=False, stop=True,
                                 skip_group_check=True)
                nc.scalar.copy(y_sb[:, n0:n0 + nsz], ps01[3 * mi + nci][:, :nsz])
            eng = nc.sync if mi % 2 == 0 else nc.scalar
            eng.dma_start(out=y[ms, :], in_=y_sb)

        # ---- m-tiles 2/3: pure main matmul stream -------------------------
        for mi in range(2, MT):
            ms = slice(mi * 128, (mi + 1) * 128)
            pss = [pspool.tile([128, 512], f32, tag="ps", name=f"ps_{mi}_{i}")
                   for i in range(len(NCHUNKS))]
            # correction FIRST (accumulation is order-free): seeds the bank
            # with start=True so no correction matmul sits in the tail
            for nci, (n0, nsz) in enumerate(NCHUNKS):
                nc.tensor.matmul(pss[nci][:, :nsz], tT33[:, ms],
                                 zs33[:, n0:n0 + nsz], start=True, stop=False,
                                 skip_group_check=True)
            for t in range(NT):
                for r in range(R):
                    last = (t == NT - 1 and r == R - 1)
                    for nci, (n0, nsz) in enumerate(NCHUNKS):
                        nc.tensor.matmul(pss[nci][:, :nsz],
                                         x_plane(xe_sb, t, r)[:, ms],
                                         w4[:, t, r, n0:n0 + nsz],
                                         start=False, stop=last,
                                         skip_group_check=True)
            y_sb = ypool.tile([128, NS], f32, tag="ysb", name=f"ysb{mi}")
            if mi == MT - 1:
                nc.scalar.copy(y_sb[:, 0:512], pss[0][:, :512])
                nc.vector.tensor_copy(y_sb[:, 512:1024], pss[1][:, :512])
                nc.sync.dma_start(out=y[ms, 0:1024], in_=y_sb[:, 0:1024])
                nc.scalar.copy(y_sb[:, 1024:NS], pss[2][:, :352])
                nc.scalar.dma_start(out=y[ms, 1024:NS], in_=y_sb[:, 1024:NS])
            else:
                for nci, (n0, nsz) in enumerate(NCHUNKS):
                    nc.scalar.copy(y_sb[:, n0:n0 + nsz], pss[nci][:, :nsz])
                eng = nc.sync if mi % 2 == 0 else nc.scalar
                eng.dma_start(out=y[ms, :], in_=y_sb)

    nc.finalize()
    return nc


def prep_in_maps(x, qweight, scales, zeros, bias):
    # x planes: xtr4[j, t, r, m] = x[m, 512t + 4j + r]
    xk = x.T.astype(np.float16)                      # [K, M]
    xtr4 = np.ascontiguousarray(
        xk.reshape(NT, 128, R, M).transpose(1, 0, 2, 3))

    E8 = np.zeros((128, NT, 32), np.float16)
    for t in range(NT):
        for j in range(128):
            E8[j, t, 4 * t + j // 32] = -1.0
    # pack x planes + indicator row into one per-tile DMA payload
    xe = np.concatenate([xtr4.reshape(128, NT, R * M), E8], axis=2)
    # srepr[j, t, n] = scalesT[4t + j//32, n]
    gi = (4 * np.arange(NT)[None, :] + np.arange(128)[:, None] // 32)  # [128, NT]

    in_maps = []
    for c in range(NCORES):
        rows = slice(c * NS, (c + 1) * NS)
        # word[i, n] = byte(kp=2i) | byte(kp=2i+1) << 8, i = 128 t + j
        qu8 = qweight[rows].astype(np.uint8).T       # [KP, NS]
        qu = qu8[0::2].astype(np.uint16) | (qu8[1::2].astype(np.uint16) << 8)
        qTr = np.ascontiguousarray(qu.reshape(NT, 128, NS).transpose(1, 0, 2))
        sT = np.ascontiguousarray(scales[rows].T)    # [32, NS]
        srepr = sT.astype(np.float16)[gi]            # [128, NT, NS] f16
        # pack words + scale-bits side by side: qsr[:, t, 0, :]=qTr, [.,1,:]=sr
        qsr = np.stack([qTr, srepr.view(np.uint16)], axis=2)  # [128, NT, 2, NS]
        in_maps.append({
            "qsr": np.ascontiguousarray(qsr),
            "xe": np.ascontiguousarray(xe),
            "sT32": sT.astype(np.float32),
            "zT32": np.ascontiguousarray(zeros[rows].T).astype(np.float32),
            "biasr": bias[rows][None, :].astype(np.float32),
        })
    return in_maps


def kernel(x, qweight, scales, zeros, bias):
    from concourse.bass_utils import run_bass_kernel_spmd

    x = np.asarray(x, dtype=np.float32)
    qweight = np.asarray(qweight)
    scales = np.asarray(scales, dtype=np.float32)
    zeros = np.asarray(zeros, dtype=np.float32)
    bias = np.asarray(bias, dtype=np.float32)

    nc = build_bass()
    in_maps = prep_in_maps(x, qweight, scales, zeros, bias)
    res = run_bass_kernel_spmd(nc, in_maps, list(range(NCORES)))
    return np.concatenate([r["y"] for r in res.results], axis=1)


# revision 11
# speedup vs baseline: 1.1748x; 1.1748x over previous
"""4-bit grouped-quant linear (BitBLAS-style) on 8 TRN2 NeuronCores.

y[m,n] = sum_k x[m,k] * (q[n,k] - zeros[n,g(k)]) * scales[n,g(k)] + bias[n]

Sharding: column-parallel (shard out_features N across 8 cores, replicate x).

Per core (N_shard = 1376), everything in [k, n] layout (host pre-transposes and
bit-packs the quantized weights — pure relayout; all math is on-device):
  - qsr[:, t, 0, :] holds packed nibble words (two packed bytes
    row-interleaved by the host), qsr[:, t, 1, :] the fp16 scales table
    replicated across the four 32-partition group bands (bit-viewed as u16 so
    both ride ONE DMA per k-tile; completion receipts are ~2us each, so fewer
    bigger DMAs win).  One [128, NS] word tile yields 4 q-planes
    (k = 4i+r) via shift/and ops; W'_r = q_r * s in fp16.
  - Dep-less warm-up matmuls run during the DMA lead-in to open the PE HAM
    clock gate before the real matmul stream arrives (any PE-idle gap >3.4us
    re-throttles the clock to half rate).
  - zero-points and bias fold into a rank-33 correction matmul:
        y = x @ (q*s)^T - sum_g zs[n,g] * t_g[m] + bias[n]
    with t_g[m] = sum_{k in g} x[m,k] via one indicator matmul per k-tile on
    xsum = sum_r x_plane_r (the adds run on the otherwise-idle GpSimd).
  - Main matmuls: lhsT = x^T plane tiles (stationary), rhs = W' tiles,
    PSUM-accumulated over 32 (t, r) k-tiles + the rank-33 correction.
  - Pipeline: the m-tiles 0+1 main matmuls are interleaved into the per-t
    dequant loop (6 PSUM banks + tps + warmup = 8), so the PE does real work
    while the k-tiles stream in from HBM; m-tiles 2+3 run after from SBUF.
  - Input DMA alternates between the two HWDGE queues (sync/scalar) per
    k-tile; the last m-tile's output store is split across both queues with
    the small chunk corrected/copied first to shorten the tail.
"""

import numpy as np
from contextlib import ExitStack

M, K, N, G = 512, 4096, 11008, 128
NCORES = 8
NS = N // NCORES          # 1376 out-features per core
NT = 8                    # uint16 word tiles (each: 128 partitions x 4 planes)
R = 4                     # nibble planes per word
MT = M // 128             # 4 m-tiles
NCHUNKS = [(0, 512), (512, 512), (1024, 352)]
NWARM = 13


def x_plane(xe_sb, t, r):
    return xe_sb[:, t, r * M:(r + 1) * M]


def build_bass():
    import concourse.mybir as mybir
    import concourse.tile as tile
    from concourse import bacc

    f16 = mybir.dt.float16
    f32 = mybir.dt.float32
    u16 = mybir.dt.uint16
    Alu = mybir.AluOpType

    nc = bacc.Bacc(None, target_bir_lowering=False)

    qsr = nc.declare_dram_parameter("qsr", [128, NT, 2, NS], u16, isOutput=False)
    xe = nc.declare_dram_parameter("xe", [128, NT, R * M + 32], f16, isOutput=False)
    sT32 = nc.declare_dram_parameter("sT32", [32, NS], f32, isOutput=False)
    zT32 = nc.declare_dram_parameter("zT32", [32, NS], f32, isOutput=False)
    biasr = nc.declare_dram_parameter("biasr", [1, NS], f32, isOutput=False)
    y = nc.declare_dram_parameter("y", [M, NS], f32, isOutput=True)

    with tile.TileContext(nc) as tc, ExitStack() as ctx:
        persist = ctx.enter_context(tc.tile_pool(name="persist", bufs=1))
        qpool = ctx.enter_context(tc.tile_pool(name="qpool", bufs=2))
        ypool = ctx.enter_context(tc.tile_pool(name="ypool", bufs=2))
        pspool = ctx.enter_context(tc.tile_pool(name="pspool", bufs=6, space="PSUM"))
        tpspool = ctx.enter_context(tc.tile_pool(name="tpspool", bufs=1, space="PSUM"))
        wupool = ctx.enter_context(tc.tile_pool(name="wupool", bufs=1, space="PSUM"))

        # ---- PE warm-up: dep-less matmuls keep the HAM clock gate open
        # while input DMAs land (PE is idle here anyway) --------------------
        wu_sb = persist.tile([128, 512], f16)
        nc.vector.memset(wu_sb, 0.0)
        wu_ps = wupool.tile([128, 512], f32)
        for i in range(NWARM):
            nc.tensor.matmul(wu_ps, wu_sb[:, :128], wu_sb,
                             start=True, stop=True, skip_group_check=True)

        # ---- per-tile: DMA, dequant, indicator + m-tile 0/1 main matmuls --
        st32_sb = persist.tile([32, NS], f32)
        zt32_sb = persist.tile([32, NS], f32)
        bias_sb = persist.tile([1, NS], f32)
        w4 = persist.tile([128, NT, R, NS], f16)
        xe_sb = persist.tile([128, NT, R * M + 32], f16)
        tps = tpspool.tile([32, M], f32)

        ps01 = [pspool.tile([128, 512], f32, tag="ps", name=f"ps_{mi}_{i}")
                for mi in range(2) for i in range(len(NCHUNKS))]
        HALF = 512
        for t in range(NT):
            qs = qpool.tile([128, 2, NS], u16, tag="qs", name=f"qs{t}")
            qt = qs[:, 0, :]
            sr = qs[:, 1, :].bitcast(f16)
            eng_q = nc.sync if t % 2 == 0 else nc.scalar
            eng_x = nc.scalar if t % 2 == 0 else nc.sync
            if t == 0:
                # halves ride both queues so dequant starts ~2us earlier
                nc.sync.dma_start(out=qs[:, :, 0:HALF], in_=qsr[:, 0, :, 0:HALF])
                nc.scalar.dma_start(out=qs[:, :, HALF:NS],
                                    in_=qsr[:, 0, :, HALF:NS])
            else:
                eng_q.dma_start(out=qs, in_=qsr[:, t, :, :])
            eng_x.dma_start(out=xe_sb[:, t, :], in_=xe[:, t, :])

            # dequant: w_r = ((qt >> 4r) & 15) * s
            pl = [qpool.tile([128, NS], u16, tag=f"pl{r}", name=f"pl{r}_{t}")
                  for r in range(R)]
            shifts = [(15, None, Alu.bitwise_and, None),
                      (4, 15, Alu.logical_shift_right, Alu.bitwise_and),
                      (8, 15, Alu.logical_shift_right, Alu.bitwise_and),
                      (12, None, Alu.logical_shift_right, None)]
            halves = [(0, HALF), (HALF, NS - HALF)] if t == 0 else [(0, NS)]
            for h0, hsz in halves:
                for r in range(R):
                    s1, s2, o1, o2 = shifts[r]
                    if o2 is None:
                        nc.vector.tensor_scalar(pl[r][:, h0:h0 + hsz],
                                                qt[:, h0:h0 + hsz], s1, None, o1)
                    else:
                        nc.vector.tensor_scalar(pl[r][:, h0:h0 + hsz],
                                                qt[:, h0:h0 + hsz], s1, s2, o1, o2)
                    nc.vector.tensor_tensor(out=w4[:, t, r, h0:h0 + hsz],
                                            in0=pl[r][:, h0:h0 + hsz],
                                            in1=sr[:, h0:h0 + hsz], op=Alu.mult)

            for r in range(R):
                nc.tensor.matmul(tps, xe_sb[:, t, R * M:R * M + 32],
                                 x_plane(xe_sb, t, r),
                                 start=(t == 0 and r == 0),
                                 stop=(t == NT - 1 and r == R - 1),
                                 skip_group_check=True)

            if t == 0:
                # chunk 0 lives in the first half; run all its mains first
                order = [(r, mi, 0) for r in range(R) for mi in range(2)] +                         [(r, mi, nci) for r in range(R) for mi in range(2)
                         for nci in (1, 2)]
            else:
                order = [(r, mi, nci) for r in range(R) for mi in range(2)
                         for nci in range(len(NCHUNKS))]
            for r, mi, nci in order:
                n0, nsz = NCHUNKS[nci]
                ms = slice(mi * 128, (mi + 1) * 128)
                nc.tensor.matmul(ps01[3 * mi + nci][:, :nsz],
                                 x_plane(xe_sb, t, r)[:, ms],
                                 w4[:, t, r, n0:n0 + nsz],
                                 start=(t == 0 and r == 0), stop=False,
                                 skip_group_check=True)

        # small tables ride the idle SWDGE queue (keeps receipts off the
        # sync/scalar chains); only needed for the correction matmuls
        nc.gpsimd.dma_start(out=st32_sb, in_=sT32[:, :])
        nc.gpsimd.dma_start(out=zt32_sb, in_=zT32[:, :])
        nc.gpsimd.dma_start(out=bias_sb, in_=biasr[:, :])

        # zs33: rows 0..31 = zeros*scales (fp32 mult -> fp16), row 32 = -bias
        zs33 = persist.tile([33, NS], f16)
        nc.vector.tensor_tensor(out=zs33[0:32, :], in0=zt32_sb, in1=st32_sb,
                                op=Alu.mult)
        nc.vector.tensor_scalar(zs33[32:33, :], bias_sb, -1.0, None, Alu.mult)

        # tT33: rows 0..31 = -t_g[m], row 32 = -1
        tT33 = persist.tile([33, M], f16)
        nc.scalar.copy(tT33[0:32, :], tps)
        nc.vector.memset(tT33[32:33, :], -1.0)

        # ---- finish m-tiles 0/1: correction + copy-out + store ------------
        for mi in range(2):
            ms = slice(mi * 128, (mi + 1) * 128)
            y_sb = ypool.tile([128, NS], f32, tag="ysb", name=f"ysb{mi}")
            for nci, (n0, nsz) in enumerate(NCHUNKS):
                nc.tensor.matmul(ps01[3 * mi + nci][:, :nsz], tT33[:, ms],
                                 zs33[:, n0:n0 + nsz], start=False, stop=True,
                                 skip_group_check=True)
                nc.scalar.copy(y_sb[:, n0:n0 + nsz], ps01[3 * mi + nci][:, :nsz])
            eng = nc.sync if mi % 2 == 0 else nc.scalar
            eng.dma_start(out=y[ms, :], in_=y_sb)

        # ---- m-tiles 2/3: pure main matmul stream -------------------------
        for mi in range(2, MT):
            ms = slice(mi * 128, (mi + 1) * 128)
            pss = [pspool.tile([128, 512], f32, tag="ps", name=f"ps_{mi}_{i}")
                   for i in range(len(NCHUNKS))]
            # correction FIRST (accumulation is order-free): seeds the bank
            # with start=True so no correction matmul sits in the tail
            for nci, (n0, nsz) in enumerate(NCHUNKS):
                nc.tensor.matmul(pss[nci][:, :nsz], tT33[:, ms],
                                 zs33[:, n0:n0 + nsz], start=True, stop=False,
                                 skip_group_check=True)
            for t in range(NT):
                for r in range(R):
                    last = (t == NT - 1 and r == R - 1)
                    for nci, (n0, nsz) in enumerate(NCHUNKS):
                        nc.tensor.matmul(pss[nci][:, :nsz],
                                         x_plane(xe_sb, t, r)[:, ms],
                                         w4[:, t, r, n0:n0 + nsz],
                                         start=False, stop=last,
                                         skip_group_check=True)
            y_sb = ypool.tile([128, NS], f32, tag="ysb", name=f"ysb{mi}")
            if mi == MT - 1:
                nc.scalar.copy(y_sb[:, 0:512], pss[0][:, :512])
                nc.vector.tensor_copy(y_sb[:, 512:1024], pss[1][:, :512])
                nc.sync.dma_start(out=y[ms, 0:1024], in_=y_sb[:, 0:1024])
                nc.scalar.copy(y_sb[:, 1024:NS], pss[2][:, :352])
                nc.scalar.dma_start(out=y[ms, 1024:NS], in_=y_sb[:, 1024:NS])
            else:
                for nci, (n0, nsz) in enumerate(NCHUNKS):
                    nc.scalar.copy(y_sb[:, n0:n0 + nsz], pss[nci][:, :nsz])
                eng = nc.sync if mi % 2 == 0 else nc.scalar
                eng.dma_start(out=y[ms, :], in_=y_sb)

    nc.finalize()
    return nc


def prep_in_maps(x, qweight, scales, zeros, bias):
    # x planes: xtr4[j, t, r, m] = x[m, 512t + 4j + r]
    xk = x.T.astype(np.float16)                      # [K, M]
    xtr4 = np.ascontiguousarray(
        xk.reshape(NT, 128, R, M).transpose(1, 0, 2, 3))

    E8 = np.zeros((128, NT, 32), np.float16)
    for t in range(NT):
        for j in range(128):
            E8[j, t, 4 * t + j // 32] = -1.0
    # pack x planes + indicator row into one per-tile DMA payload
    xe = np.concatenate([xtr4.reshape(128, NT, R * M), E8], axis=2)
    # srepr[j, t, n] = scalesT[4t + j//32, n]
    gi = (4 * np.arange(NT)[None, :] + np.arange(128)[:, None] // 32)  # [128, NT]

    in_maps = []
    for c in range(NCORES):
        rows = slice(c * NS, (c + 1) * NS)
        # word[i, n] = byte(kp=2i) | byte(kp=2i+1) << 8, i = 128 t + j
        qu8 = qweight[rows].astype(np.uint8).T       # [KP, NS]
        qu = qu8[0::2].astype(np.uint16) | (qu8[1::2].astype(np.uint16) << 8)
        qTr = np.ascontiguousarray(qu.reshape(NT, 128, NS).transpose(1, 0, 2))
        sT = np.ascontiguousarray(scales[rows].T)    # [32, NS]
        srepr = sT.astype(np.float16)[gi]            # [128, NT, NS] f16
        # pack words + scale-bits side by side: qsr[:, t, 0, :]=qTr, [.,1,:]=sr
        qsr = np.stack([qTr, srepr.view(np.uint16)], axis=2)  # [128, NT, 2, NS]
        in_maps.append({
            "qsr": np.ascontiguousarray(qsr),
            "xe": np.ascontiguousarray(xe),
            "sT32": sT.astype(np.float32),
            "zT32": np.ascontiguousarray(zeros[rows].T).astype(np.float32),
            "biasr": bias[rows][None, :].astype(np.float32),
        })
    return in_maps


def kernel(x, qweight, scales, zeros, bias):
    from concourse.bass_utils import run_bass_kernel_spmd

    x = np.asarray(x, dtype=np.float32)
    qweight = np.asarray(qweight)
    scales = np.asarray(scales, dtype=np.float32)
    zeros = np.asarray(zeros, dtype=np.float32)
    bias = np.asarray(bias, dtype=np.float32)

    nc = build_bass()
    in_maps = prep_in_maps(x, qweight, scales, zeros, bias)
    res = run_bass_kernel_spmd(nc, in_maps, list(range(NCORES)))
    return np.concatenate([r["y"] for r in res.results], axis=1)
